# revision 34
# baseline (speedup 1.0000x reference)
"""RNN-T Joiner kernel for Trainium2, data-parallel over (B, T) on 8 cores.

reference:
    logit = tanh(enc[:, :, None, :] + dec[:, None, :, :])   # (B,T,U,C)
    out   = einsum('btuc,vc->btuv', logit, W) + b           # (B,T,U,V)

Shapes (hardcoded): B=4, T=256, U=64, C=512, V=1024.

Sharding: core k handles b = k//2, t rows [ (k%2)*128, (k%2)*128+128 ).
W / bias replicated. No collectives.

Per-core device kernel (C on partitions for the logit):
  - logitT[c, t] = tanh(encT[c, t] + decT[c, u])  -- scalar engine, fused
    per-partition bias add, fp16 out; tanh LUT preloaded by a dummy
    activation during the DMA head.
  - out[t, v] accumulated over 4 c-chunks of K=128 fp16 matmuls (full PE
    rate). PE pre-warmed with dummy matmuls so the DVFS ramp is spent
    before real work arrives.
  - DMA-queue descriptor cadence dominates the head, so inputs minimize
    descriptor count: enc+dec fused into one contiguous param on the SP
    queues; W split per c-chunk across the SP and Activation-engine
    queue sets so both descriptor generators run in parallel.
  - first 4 u-steps run c-major (cross-u interleaved) so the PE chews
    through chunk-0 matmuls while the remaining W chunks stream in.
  - bias broadcast on-chip from a 4KB row via a K=1 matmul; bias add
    fused into the PSUM->SBUF eviction on DVE.
  - output fp16 (host upcasts), 2 u-steps per DMA; final u split into
    half-width evict+DMA pairs to shorten the tail.
"""

import numpy as np

B, T, U, C, V = 4, 256, 64, 512, 1024
NCORES = 8
TS = 128  # t rows per core
CCH = C // 128  # 4 contraction chunks
VH = V // 512  # 2 psum-width chunks
UB = 2  # u-steps batched per output DMA
NI = 4  # leading u-steps run c-major (cross-u interleaved)
NWARM_BIG = 5  # 512-row warmup matmuls (DVFS ramp)
NWARM_SMALL = 14  # 128-row warmup matmuls (>=128 rows or the clock drops)
ED_F32 = CCH * TS + CCH * U  # enc + dec fused rows

_CACHE = {}


def _build():
    from contextlib import ExitStack

    import concourse.bacc as bacc
    import concourse.mybir as mybir
    import concourse.tile as tile

    dt = mybir.dt
    f32 = dt.float32
    f16 = dt.float16

    nc = bacc.Bacc("TRN2", target_bir_lowering=False, debug=False, num_devices=NCORES)
    # ed: per-partition-row = enc(512 f32) | dec(256 f32)
    ed = nc.declare_dram_parameter("ed", [128, ED_F32], f32, isOutput=False)
    wt = nc.declare_dram_parameter("wt", [128, CCH * V], f16, isOutput=False)
    bias_row = nc.declare_dram_parameter("bias_row", [1, V], f16, isOutput=False)
    out = nc.declare_dram_parameter("out", [TS, U, V], f16, isOutput=True)

    with tile.TileContext(nc) as tc, ExitStack() as ctx:
        const = ctx.enter_context(tc.tile_pool(name="const", bufs=1))
        logit_pool = ctx.enter_context(tc.tile_pool(name="logit", bufs=6))
        psum_pool = ctx.enter_context(tc.tile_pool(name="psum", bufs=4, space="PSUM"))
        out_pool = ctx.enter_context(tc.tile_pool(name="out", bufs=3))

        warm_sb = const.tile([128, 512], f16, tag="warm")
        ones_sb = const.tile([1, 128], f16, tag="ones")
        scr_sb = const.tile([128, 4], f16, tag="scr")
        ed_sb = const.tile([128, ED_F32], f32, tag="ed")
        wt_sb = const.tile([128, CCH * V], f16, tag="wt")
        brow_sb = const.tile([1, V], f16, tag="brow")
        bias_sb = const.tile([128, V], f32, tag="bias")

        enc_v = ed_sb[:, 0 : CCH * TS]
        dec_v = ed_sb[:, CCH * TS : ED_F32]

        def wt_chunk(c, vh):
            off = c * V + vh * 512
            return wt_sb[:, off : off + 512]

        nc.vector.memset(warm_sb[:], 0.0)
        nc.vector.memset(ones_sb[:], 1.0)
        # dummy activation: loads the tanh LUT while input DMAs stream
        nc.scalar.activation(
            scr_sb[:], warm_sb[:, :4], mybir.ActivationFunctionType.Tanh
        )

        # input DMAs split across both HWDGE engine queue sets so the two
        # descriptor generators run in parallel
        nc.sync.dma_start(ed_sb[:], ed[:])
        nc.sync.dma_start(wt_sb[:, 0:V], wt[:, 0:V])
        nc.sync.dma_start(wt_sb[:, V : 2 * V], wt[:, V : 2 * V])
        nc.scalar.dma_start(brow_sb[:], bias_row[:])
        nc.scalar.dma_start(wt_sb[:, 2 * V : 3 * V], wt[:, 2 * V : 3 * V])
        nc.scalar.dma_start(wt_sb[:, 3 * V : 4 * V], wt[:, 3 * V : 4 * V])

        # PE warmup: no data deps, runs during the DMA head
        for _ in range(NWARM_BIG):
            wp = psum_pool.tile([128, V], f32, tag="ps", name="wp")
            nc.tensor.matmul(
                wp[:, :512],
                lhsT=warm_sb[:, :128],
                rhs=warm_sb[:],
                start=True,
                stop=True,
            )
        for _ in range(NWARM_SMALL):
            wp = psum_pool.tile([128, V], f32, tag="ps", name="wp")
            nc.tensor.matmul(
                wp[:, :128],
                lhsT=warm_sb[:, :128],
                rhs=warm_sb[:, :128],
                start=True,
                stop=True,
            )

        # broadcast bias row across partitions: ones[1,128].T @ brow[1,512]
        for vh in range(VH):
            bp = psum_pool.tile([128, V], f32, tag="ps", name="bp")
            nc.tensor.matmul(
                bp[:, :512],
                lhsT=ones_sb[:],
                rhs=brow_sb[:, vh * 512 : (vh + 1) * 512],
                start=True,
                stop=True,
            )
            nc.vector.tensor_scalar_add(
                bias_sb[:, vh * 512 : (vh + 1) * 512], bp[:, :512], 0.0
            )

        def make_lg(u):
            lgs = []
            for c in range(CCH):
                lg_c = logit_pool.tile([128, TS], f16, tag=f"lg{c}", name=f"lg{c}")
                lgs.append(lg_c)
            return lgs

        def act(lgs, u, c):
            nc.scalar.activation(
                lgs[c][:],
                enc_v[:, c * TS : (c + 1) * TS],
                mybir.ActivationFunctionType.Tanh,
                bias=dec_v[:, c * U + u : c * U + u + 1],
            )

        def mm(ps, lgs, c, vh):
            nc.tensor.matmul(
                ps[:, vh * 512 : (vh + 1) * 512],
                lhsT=lgs[c][:],
                rhs=wt_chunk(c, vh),
                start=(c == 0),
                stop=(c == CCH - 1),
            )

        def evict(ob, ps, u, j, last):
            if last and j == UB - 1:
                # split the final eviction: tail interleaves evict/DMA, and
                # the two half-DMAs ride separate HWDGE queue sets
                for vh in range(VH):
                    sl = slice(j * V + vh * 512, j * V + (vh + 1) * 512)
                    nc.vector.tensor_add(
                        ob[:, sl],
                        ps[:, vh * 512 : (vh + 1) * 512],
                        bias_sb[:, vh * 512 : (vh + 1) * 512],
                    )
                    eng = nc.sync if vh == 0 else nc.scalar
                    eng.dma_start(out[:, u, vh * 512 : (vh + 1) * 512], ob[:, sl])
            else:
                nc.vector.tensor_add(ob[:, j * V : (j + 1) * V], ps[:], bias_sb[:])
                if last:
                    nc.scalar.dma_start(out[:, u, :], ob[:, j * V : (j + 1) * V])

        # ---- leading NI u-steps, c-major so chunk-0 matmuls run while
        # wt_rest streams in ----
        lead_lgs = [make_lg(u) for u in range(NI)]
        lead_ps = []
        for u in range(NI):
            ps = psum_pool.tile([128, V], f32, tag="ps", name="ps")
            lead_ps.append(ps)
        lead_obs = [out_pool.tile([128, UB * V], f16, tag="ob", name="ob")
                    for _ in range(NI // UB)]
        for c in range(CCH):
            for u in range(NI):
                act(lead_lgs[u], u, c)
        for c in range(CCH):
            for u in range(NI):
                for vh in range(VH):
                    mm(lead_ps[u], lead_lgs[u], c, vh)
        for u in range(NI):
            evict(lead_obs[u // UB], lead_ps[u], u, u % UB, False)
        for i, ob in enumerate(lead_obs):
            nc.sync.dma_start(out[:, i * UB : (i + 1) * UB, :], ob[:])

        # ---- steady state ----
        for ub in range(NI // UB, U // UB):
            last = ub == U // UB - 1
            ob = out_pool.tile([128, UB * V], f16, tag="ob")
            for j in range(UB):
                u = ub * UB + j
                lgs = make_lg(u)
                for c in range(CCH):
                    act(lgs, u, c)
                ps = psum_pool.tile([128, V], f32, tag="ps")
                if last and j == UB - 1:
                    # vh-major so the vh=0 accumulation finishes 4 matmuls
                    # early and its eviction overlaps the final matmuls
                    for vh in range(VH):
                        for c in range(CCH):
                            mm(ps, lgs, c, vh)
                else:
                    for c in range(CCH):
                        for vh in range(VH):
                            mm(ps, lgs, c, vh)
                evict(ob, ps, u, j, last)
            if not last:
                nc.sync.dma_start(out[:, ub * UB : (ub + 1) * UB, :], ob[:])

    nc.finalize()
    return nc


def _get_nc():
    if "nc" not in _CACHE:
        _CACHE["nc"] = _build()
    return _CACHE["nc"]


def _chunked(x):
    # [C, N] -> [128, CCH*N] with row p, col c*N+n = x[c*128+p, n]
    n = x.shape[1]
    return np.ascontiguousarray(
        x.reshape(CCH, 128, n).transpose(1, 0, 2).reshape(128, CCH * n)
    )


def kernel(**inputs):
    enc = np.asarray(inputs["enc_out"], dtype=np.float32)
    dec = np.asarray(inputs["dec_out"], dtype=np.float32)
    W = np.asarray(inputs["W"], dtype=np.float32)
    b = np.asarray(inputs["b"], dtype=np.float32)

    nc = _get_nc()

    wt_np = _chunked(W.T.astype(np.float32)).astype(np.float16)  # [128, CCH*V] f16
    brow_np = np.ascontiguousarray(b.reshape(1, V)).astype(np.float16)
    in_maps = []
    for k in range(NCORES):
        bb, t0 = k // 2, (k % 2) * TS
        enc_l = _chunked(np.ascontiguousarray(enc[bb, t0 : t0 + TS, :].T))
        dec_l = _chunked(np.ascontiguousarray(dec[bb].T))
        in_maps.append(
            {
                "ed": np.ascontiguousarray(np.concatenate([enc_l, dec_l], axis=1)),
                "wt": wt_np,
                "bias_row": brow_np,
            }
        )

    from concourse.bass_utils import run_bass_kernel_spmd

    res = run_bass_kernel_spmd(nc, in_maps, list(range(NCORES)))
    _CACHE["last_result"] = res

    out = np.empty((B, T, U, V), np.float32)
    for k in range(NCORES):
        bb, t0 = k // 2, (k % 2) * TS
        out[bb, t0 : t0 + TS] = res.results[k]["out"].astype(np.float32)
    return out


# revision 35
# speedup vs baseline: 1.0236x; 1.0236x over previous
"""RNN-T Joiner kernel for Trainium2, data-parallel over (B, T) on 8 cores.

reference:
    logit = tanh(enc[:, :, None, :] + dec[:, None, :, :])   # (B,T,U,C)
    out   = einsum('btuc,vc->btuv', logit, W) + b           # (B,T,U,V)

Shapes (hardcoded): B=4, T=256, U=64, C=512, V=1024.

Sharding: core k handles b = k//2, t rows [ (k%2)*128, (k%2)*128+128 ).
W / bias replicated. No collectives.

Per-core device kernel (C on partitions for the logit):
  - logitT[c, t] = tanh(encT[c, t] + decT[c, u])  -- scalar engine, fused
    per-partition bias add, fp16 out; tanh LUT preloaded by a dummy
    activation during the DMA head.
  - out[t, v] accumulated over 4 c-chunks of K=128 fp16 matmuls (full PE
    rate). PE pre-warmed with dummy matmuls so the DVFS ramp is spent
    before real work arrives.
  - DMA-queue descriptor cadence dominates the head, so inputs minimize
    descriptor count: enc+dec fused into one contiguous param on the SP
    queues; W split per c-chunk across the SP and Activation-engine
    queue sets so both descriptor generators run in parallel.
  - first 4 u-steps run c-major (cross-u interleaved) so the PE chews
    through chunk-0 matmuls while the remaining W chunks stream in.
  - bias broadcast on-chip from a 4KB row via a K=1 matmul; bias add
    fused into the PSUM->SBUF eviction on DVE.
  - output fp16 (host upcasts), 2 u-steps per DMA; final u split into
    half-width evict+DMA pairs to shorten the tail.
"""

import numpy as np

B, T, U, C, V = 4, 256, 64, 512, 1024
NCORES = 8
TS = 128  # t rows per core
CCH = C // 128  # 4 contraction chunks
VH = V // 512  # 2 psum-width chunks
UB = 2  # u-steps batched per output DMA
NI = 4  # leading u-steps run c-major (cross-u interleaved)
NWARM_BIG = 5  # 512-row warmup matmuls (DVFS ramp)
NWARM_SMALL = 14  # 128-row warmup matmuls (>=128 rows or the clock drops)
ED_F32 = CCH * TS + CCH * U  # enc + dec fused rows

_CACHE = {}


def _build():
    from contextlib import ExitStack

    import concourse.bacc as bacc
    import concourse.mybir as mybir
    import concourse.tile as tile

    dt = mybir.dt
    f32 = dt.float32
    f16 = dt.float16

    nc = bacc.Bacc("TRN2", target_bir_lowering=False, debug=False, num_devices=NCORES)
    # ed: per-partition-row = enc(512 f32) | dec(256 f32)
    ed = nc.declare_dram_parameter("ed", [128, ED_F32], f32, isOutput=False)
    wt = nc.declare_dram_parameter("wt", [128, CCH * V], f16, isOutput=False)
    bias_row = nc.declare_dram_parameter("bias_row", [1, V], f16, isOutput=False)
    out = nc.declare_dram_parameter("out", [TS, U, V], f16, isOutput=True)

    with tile.TileContext(nc) as tc, ExitStack() as ctx:
        const = ctx.enter_context(tc.tile_pool(name="const", bufs=1))
        logit_pool = ctx.enter_context(tc.tile_pool(name="logit", bufs=6))
        psum_pool = ctx.enter_context(tc.tile_pool(name="psum", bufs=4, space="PSUM"))
        out_pool = ctx.enter_context(tc.tile_pool(name="out", bufs=3))

        warm_sb = const.tile([128, 512], f16, tag="warm")
        ones_sb = const.tile([1, 128], f16, tag="ones")
        scr_sb = const.tile([128, 4], f16, tag="scr")
        ed_sb = const.tile([128, ED_F32], f32, tag="ed")
        wt_sb = const.tile([128, CCH * V], f16, tag="wt")
        brow_sb = const.tile([1, V], f16, tag="brow")
        bias_sb = const.tile([128, V], f32, tag="bias")

        enc_v = ed_sb[:, 0 : CCH * TS]
        dec_v = ed_sb[:, CCH * TS : ED_F32]

        def wt_chunk(c, vh):
            off = c * V + vh * 512
            return wt_sb[:, off : off + 512]

        nc.vector.memset(warm_sb[:], 0.0)
        nc.vector.memset(ones_sb[:], 1.0)
        # dummy activation: loads the tanh LUT while input DMAs stream
        nc.scalar.activation(
            scr_sb[:], warm_sb[:, :4], mybir.ActivationFunctionType.Tanh
        )

        # input DMAs split across both HWDGE engine queue sets so the two
        # descriptor generators run in parallel
        nc.sync.dma_start(ed_sb[:], ed[:])
        nc.sync.dma_start(wt_sb[:, 0:V], wt[:, 0:V])
        nc.sync.dma_start(wt_sb[:, V : 2 * V], wt[:, V : 2 * V])
        nc.scalar.dma_start(brow_sb[:], bias_row[:])
        nc.sync.dma_start(wt_sb[:, 2 * V : 3 * V], wt[:, 2 * V : 3 * V])
        nc.sync.dma_start(wt_sb[:, 3 * V : 4 * V], wt[:, 3 * V : 4 * V])

        # PE warmup: no data deps, runs during the DMA head
        for _ in range(NWARM_BIG):
            wp = psum_pool.tile([128, V], f32, tag="ps", name="wp")
            nc.tensor.matmul(
                wp[:, :512],
                lhsT=warm_sb[:, :128],
                rhs=warm_sb[:],
                start=True,
                stop=True,
            )
        for _ in range(NWARM_SMALL):
            wp = psum_pool.tile([128, V], f32, tag="ps", name="wp")
            nc.tensor.matmul(
                wp[:, :128],
                lhsT=warm_sb[:, :128],
                rhs=warm_sb[:, :128],
                start=True,
                stop=True,
            )

        # broadcast bias row across partitions: ones[1,128].T @ brow[1,512]
        for vh in range(VH):
            bp = psum_pool.tile([128, V], f32, tag="ps", name="bp")
            nc.tensor.matmul(
                bp[:, :512],
                lhsT=ones_sb[:],
                rhs=brow_sb[:, vh * 512 : (vh + 1) * 512],
                start=True,
                stop=True,
            )
            nc.vector.tensor_scalar_add(
                bias_sb[:, vh * 512 : (vh + 1) * 512], bp[:, :512], 0.0
            )

        def make_lg(u):
            lgs = []
            for c in range(CCH):
                lg_c = logit_pool.tile([128, TS], f16, tag=f"lg{c}", name=f"lg{c}")
                lgs.append(lg_c)
            return lgs

        def act(lgs, u, c):
            nc.scalar.activation(
                lgs[c][:],
                enc_v[:, c * TS : (c + 1) * TS],
                mybir.ActivationFunctionType.Tanh,
                bias=dec_v[:, c * U + u : c * U + u + 1],
            )

        def mm(ps, lgs, c, vh):
            nc.tensor.matmul(
                ps[:, vh * 512 : (vh + 1) * 512],
                lhsT=lgs[c][:],
                rhs=wt_chunk(c, vh),
                start=(c == 0),
                stop=(c == CCH - 1),
            )

        def evict(ob, ps, u, j, last):
            if last and j == UB - 1:
                # split the final eviction: tail interleaves evict/DMA, and
                # the two half-DMAs ride separate HWDGE queue sets
                for vh in range(VH):
                    sl = slice(j * V + vh * 512, j * V + (vh + 1) * 512)
                    nc.vector.tensor_add(
                        ob[:, sl],
                        ps[:, vh * 512 : (vh + 1) * 512],
                        bias_sb[:, vh * 512 : (vh + 1) * 512],
                    )
                    eng = nc.sync if vh == 0 else nc.scalar
                    eng.dma_start(out[:, u, vh * 512 : (vh + 1) * 512], ob[:, sl])
            else:
                nc.vector.tensor_add(ob[:, j * V : (j + 1) * V], ps[:], bias_sb[:])
                if last:
                    nc.scalar.dma_start(out[:, u, :], ob[:, j * V : (j + 1) * V])

        # ---- leading NI u-steps, c-major so chunk-0 matmuls run while
        # wt_rest streams in ----
        lead_lgs = [make_lg(u) for u in range(NI)]
        lead_ps = []
        for u in range(NI):
            ps = psum_pool.tile([128, V], f32, tag="ps", name="ps")
            lead_ps.append(ps)
        lead_obs = [out_pool.tile([128, UB * V], f16, tag="ob", name="ob")
                    for _ in range(NI // UB)]
        for c in range(CCH):
            for u in range(NI):
                act(lead_lgs[u], u, c)
        for c in range(CCH):
            for u in range(NI):
                for vh in range(VH):
                    mm(lead_ps[u], lead_lgs[u], c, vh)
        for u in range(NI):
            evict(lead_obs[u // UB], lead_ps[u], u, u % UB, False)
        for i, ob in enumerate(lead_obs):
            nc.sync.dma_start(out[:, i * UB : (i + 1) * UB, :], ob[:])

        # ---- steady state ----
        for ub in range(NI // UB, U // UB):
            last = ub == U // UB - 1
            ob = out_pool.tile([128, UB * V], f16, tag="ob")
            for j in range(UB):
                u = ub * UB + j
                lgs = make_lg(u)
                for c in range(CCH):
                    act(lgs, u, c)
                ps = psum_pool.tile([128, V], f32, tag="ps")
                if last and j == UB - 1:
                    # vh-major so the vh=0 accumulation finishes 4 matmuls
                    # early and its eviction overlaps the final matmuls
                    for vh in range(VH):
                        for c in range(CCH):
                            mm(ps, lgs, c, vh)
                else:
                    for c in range(CCH):
                        for vh in range(VH):
                            mm(ps, lgs, c, vh)
                evict(ob, ps, u, j, last)
            if not last:
                nc.sync.dma_start(out[:, ub * UB : (ub + 1) * UB, :], ob[:])

    nc.finalize()
    return nc


def _get_nc():
    if "nc" not in _CACHE:
        _CACHE["nc"] = _build()
    return _CACHE["nc"]


def _chunked(x):
    # [C, N] -> [128, CCH*N] with row p, col c*N+n = x[c*128+p, n]
    n = x.shape[1]
    return np.ascontiguousarray(
        x.reshape(CCH, 128, n).transpose(1, 0, 2).reshape(128, CCH * n)
    )


def kernel(**inputs):
    enc = np.asarray(inputs["enc_out"], dtype=np.float32)
    dec = np.asarray(inputs["dec_out"], dtype=np.float32)
    W = np.asarray(inputs["W"], dtype=np.float32)
    b = np.asarray(inputs["b"], dtype=np.float32)

    nc = _get_nc()

    wt_np = _chunked(W.T.astype(np.float32)).astype(np.float16)  # [128, CCH*V] f16
    brow_np = np.ascontiguousarray(b.reshape(1, V)).astype(np.float16)
    in_maps = []
    for k in range(NCORES):
        bb, t0 = k // 2, (k % 2) * TS
        enc_l = _chunked(np.ascontiguousarray(enc[bb, t0 : t0 + TS, :].T))
        dec_l = _chunked(np.ascontiguousarray(dec[bb].T))
        in_maps.append(
            {
                "ed": np.ascontiguousarray(np.concatenate([enc_l, dec_l], axis=1)),
                "wt": wt_np,
                "bias_row": brow_np,
            }
        )

    from concourse.bass_utils import run_bass_kernel_spmd

    res = run_bass_kernel_spmd(nc, in_maps, list(range(NCORES)))
    _CACHE["last_result"] = res

    out = np.empty((B, T, U, V), np.float32)
    for k in range(NCORES):
        bb, t0 = k // 2, (k % 2) * TS
        out[bb, t0 : t0 + TS] = res.results[k]["out"].astype(np.float32)
    return out


# revision 36
# speedup vs baseline: 1.0436x; 1.0195x over previous
"""RNN-T Joiner kernel for Trainium2, data-parallel over (B, T) on 8 cores.

reference:
    logit = tanh(enc[:, :, None, :] + dec[:, None, :, :])   # (B,T,U,C)
    out   = einsum('btuc,vc->btuv', logit, W) + b           # (B,T,U,V)

Shapes (hardcoded): B=4, T=256, U=64, C=512, V=1024.

Sharding: core k handles b = k//2, t rows [ (k%2)*128, (k%2)*128+128 ).
W / bias replicated. No collectives.

Per-core device kernel (C on partitions for the logit):
  - logitT[c, t] = tanh(encT[c, t] + decT[c, u])  -- scalar engine, fused
    per-partition bias add, fp16 out; tanh LUT preloaded by a dummy
    activation during the DMA head.
  - out[t, v] accumulated over 4 c-chunks of K=128 fp16 matmuls (full PE
    rate). PE pre-warmed with dummy matmuls so the DVFS ramp is spent
    before real work arrives.
  - DMA-queue descriptor cadence dominates the head, so inputs minimize
    descriptor count: enc+dec fused into one contiguous param on the SP
    queues; W split per c-chunk across the SP and Activation-engine
    queue sets so both descriptor generators run in parallel.
  - first 4 u-steps run c-major (cross-u interleaved) so the PE chews
    through chunk-0 matmuls while the remaining W chunks stream in.
  - bias broadcast on-chip from a 4KB row via a K=1 matmul; bias add
    fused into the PSUM->SBUF eviction on DVE.
  - output fp16 (host upcasts), 2 u-steps per DMA; final u split into
    half-width evict+DMA pairs to shorten the tail.
"""

import numpy as np

B, T, U, C, V = 4, 256, 64, 512, 1024
NCORES = 8
TS = 128  # t rows per core
CCH = C // 128  # 4 contraction chunks
VH = V // 512  # 2 psum-width chunks
UB = 2  # u-steps batched per output DMA
NI = 4  # leading u-steps run c-major (cross-u interleaved)
NWARM_BIG = 5  # 512-row warmup matmuls (DVFS ramp)
NWARM_SMALL = 14  # 128-row warmup matmuls (>=128 rows or the clock drops)
ED_F32 = CCH * TS + CCH * U  # enc + dec fused rows

_CACHE = {}


def _build():
    from contextlib import ExitStack

    import concourse.bacc as bacc
    import concourse.mybir as mybir
    import concourse.tile as tile

    dt = mybir.dt
    f32 = dt.float32
    f16 = dt.float16

    nc = bacc.Bacc("TRN2", target_bir_lowering=False, debug=False, num_devices=NCORES)
    # ed: per-partition-row = enc(512 f32) | dec(256 f32)
    ed = nc.declare_dram_parameter("ed", [128, ED_F32], f32, isOutput=False)
    wt = nc.declare_dram_parameter("wt", [128, CCH * V], f16, isOutput=False)
    bias_row = nc.declare_dram_parameter("bias_row", [1, V], f16, isOutput=False)
    out = nc.declare_dram_parameter("out", [TS, U, V], f16, isOutput=True)

    with tile.TileContext(nc) as tc, ExitStack() as ctx:
        const = ctx.enter_context(tc.tile_pool(name="const", bufs=1))
        logit_pool = ctx.enter_context(tc.tile_pool(name="logit", bufs=6))
        psum_pool = ctx.enter_context(tc.tile_pool(name="psum", bufs=4, space="PSUM"))
        out_pool = ctx.enter_context(tc.tile_pool(name="out", bufs=3))

        warm_sb = const.tile([128, 512], f16, tag="warm")
        ones_sb = const.tile([1, 128], f16, tag="ones")
        scr_sb = const.tile([128, 4], f16, tag="scr")
        ed_sb = const.tile([128, ED_F32], f32, tag="ed")
        wt_sb = const.tile([128, CCH * V], f16, tag="wt")
        brow_sb = const.tile([1, V], f16, tag="brow")
        bias_sb = const.tile([128, V], f32, tag="bias")

        enc_v = ed_sb[:, 0 : CCH * TS]
        dec_v = ed_sb[:, CCH * TS : ED_F32]

        def wt_chunk(c, vh):
            off = c * V + vh * 512
            return wt_sb[:, off : off + 512]

        nc.vector.memset(warm_sb[:], 0.0)
        nc.vector.memset(ones_sb[:], 1.0)
        # dummy activation: loads the tanh LUT while input DMAs stream
        nc.scalar.activation(
            scr_sb[:], warm_sb[:, :4], mybir.ActivationFunctionType.Tanh
        )

        # input DMAs split across both HWDGE engine queue sets so the two
        # descriptor generators run in parallel
        nc.sync.dma_start(ed_sb[:], ed[:])
        nc.sync.dma_start(wt_sb[:, 0:V], wt[:, 0:V])
        nc.sync.dma_start(wt_sb[:, V : 2 * V], wt[:, V : 2 * V])
        nc.scalar.dma_start(brow_sb[:], bias_row[:])
        nc.sync.dma_start(wt_sb[:, 2 * V : 3 * V], wt[:, 2 * V : 3 * V])
        nc.sync.dma_start(wt_sb[:, 3 * V : 4 * V], wt[:, 3 * V : 4 * V])

        # PE warmup: no data deps, runs during the DMA head
        for _ in range(NWARM_BIG):
            wp = psum_pool.tile([128, V], f32, tag="ps", name="wp")
            nc.tensor.matmul(
                wp[:, :512],
                lhsT=warm_sb[:, :128],
                rhs=warm_sb[:],
                start=True,
                stop=True,
            )
        for _ in range(NWARM_SMALL):
            wp = psum_pool.tile([128, V], f32, tag="ps", name="wp")
            nc.tensor.matmul(
                wp[:, :128],
                lhsT=warm_sb[:, :128],
                rhs=warm_sb[:, :128],
                start=True,
                stop=True,
            )

        # broadcast bias row across partitions: ones[1,128].T @ brow[1,512]
        for vh in range(VH):
            bp = psum_pool.tile([128, V], f32, tag="ps", name="bp")
            nc.tensor.matmul(
                bp[:, :512],
                lhsT=ones_sb[:],
                rhs=brow_sb[:, vh * 512 : (vh + 1) * 512],
                start=True,
                stop=True,
            )
            nc.vector.tensor_scalar_add(
                bias_sb[:, vh * 512 : (vh + 1) * 512], bp[:, :512], 0.0
            )

        def make_lg(u):
            lgs = []
            for c in range(CCH):
                lg_c = logit_pool.tile([128, TS], f16, tag=f"lg{c}", name=f"lg{c}")
                lgs.append(lg_c)
            return lgs

        def act(lgs, u, c):
            nc.scalar.activation(
                lgs[c][:],
                enc_v[:, c * TS : (c + 1) * TS],
                mybir.ActivationFunctionType.Tanh,
                bias=dec_v[:, c * U + u : c * U + u + 1],
            )

        def mm(ps, lgs, c, vh):
            nc.tensor.matmul(
                ps[:, vh * 512 : (vh + 1) * 512],
                lhsT=lgs[c][:],
                rhs=wt_chunk(c, vh),
                start=(c == 0),
                stop=(c == CCH - 1),
            )

        def evict(ob, ps, u, j, last):
            if last and j == UB - 1:
                # split the final eviction: tail interleaves evict/DMA, and
                # each half-DMA is further split by partition rows across
                # both HWDGE queue sets (64+64 descriptors in parallel)
                for vh in range(VH):
                    sl = slice(j * V + vh * 512, j * V + (vh + 1) * 512)
                    nc.vector.tensor_add(
                        ob[:, sl],
                        ps[:, vh * 512 : (vh + 1) * 512],
                        bias_sb[:, vh * 512 : (vh + 1) * 512],
                    )
                    vsl = slice(vh * 512, (vh + 1) * 512)
                    nc.sync.dma_start(out[0:64, u, vsl], ob[0:64, sl])
                    nc.scalar.dma_start(out[64:128, u, vsl], ob[64:128, sl])
            else:
                nc.vector.tensor_add(ob[:, j * V : (j + 1) * V], ps[:], bias_sb[:])
                if last:
                    nc.sync.dma_start(out[:, u, :], ob[:, j * V : (j + 1) * V])

        # ---- leading NI u-steps, c-major so chunk-0 matmuls run while
        # wt_rest streams in ----
        lead_lgs = [make_lg(u) for u in range(NI)]
        lead_ps = []
        for u in range(NI):
            ps = psum_pool.tile([128, V], f32, tag="ps", name="ps")
            lead_ps.append(ps)
        lead_obs = [out_pool.tile([128, UB * V], f16, tag="ob", name="ob")
                    for _ in range(NI // UB)]
        for c in range(CCH):
            for u in range(NI):
                act(lead_lgs[u], u, c)
        for c in range(CCH):
            for u in range(NI):
                for vh in range(VH):
                    mm(lead_ps[u], lead_lgs[u], c, vh)
        for u in range(NI):
            evict(lead_obs[u // UB], lead_ps[u], u, u % UB, False)
        for i, ob in enumerate(lead_obs):
            nc.sync.dma_start(out[:, i * UB : (i + 1) * UB, :], ob[:])

        # ---- steady state ----
        for ub in range(NI // UB, U // UB):
            last = ub == U // UB - 1
            ob = out_pool.tile([128, UB * V], f16, tag="ob")
            for j in range(UB):
                u = ub * UB + j
                lgs = make_lg(u)
                for c in range(CCH):
                    act(lgs, u, c)
                ps = psum_pool.tile([128, V], f32, tag="ps")
                if last and j == UB - 1:
                    # vh-major so the vh=0 accumulation finishes 4 matmuls
                    # early and its eviction overlaps the final matmuls
                    for vh in range(VH):
                        for c in range(CCH):
                            mm(ps, lgs, c, vh)
                else:
                    for c in range(CCH):
                        for vh in range(VH):
                            mm(ps, lgs, c, vh)
                evict(ob, ps, u, j, last)
            if not last:
                nc.sync.dma_start(out[:, ub * UB : (ub + 1) * UB, :], ob[:])

    nc.finalize()
    return nc


def _get_nc():
    if "nc" not in _CACHE:
        _CACHE["nc"] = _build()
    return _CACHE["nc"]


def _chunked(x):
    # [C, N] -> [128, CCH*N] with row p, col c*N+n = x[c*128+p, n]
    n = x.shape[1]
    return np.ascontiguousarray(
        x.reshape(CCH, 128, n).transpose(1, 0, 2).reshape(128, CCH * n)
    )


def kernel(**inputs):
    enc = np.asarray(inputs["enc_out"], dtype=np.float32)
    dec = np.asarray(inputs["dec_out"], dtype=np.float32)
    W = np.asarray(inputs["W"], dtype=np.float32)
    b = np.asarray(inputs["b"], dtype=np.float32)

    nc = _get_nc()

    wt_np = _chunked(W.T.astype(np.float32)).astype(np.float16)  # [128, CCH*V] f16
    brow_np = np.ascontiguousarray(b.reshape(1, V)).astype(np.float16)
    in_maps = []
    for k in range(NCORES):
        bb, t0 = k // 2, (k % 2) * TS
        enc_l = _chunked(np.ascontiguousarray(enc[bb, t0 : t0 + TS, :].T))
        dec_l = _chunked(np.ascontiguousarray(dec[bb].T))
        in_maps.append(
            {
                "ed": np.ascontiguousarray(np.concatenate([enc_l, dec_l], axis=1)),
                "wt": wt_np,
                "bias_row": brow_np,
            }
        )

    from concourse.bass_utils import run_bass_kernel_spmd

    res = run_bass_kernel_spmd(nc, in_maps, list(range(NCORES)))
    _CACHE["last_result"] = res

    out = np.empty((B, T, U, V), np.float32)
    for k in range(NCORES):
        bb, t0 = k // 2, (k % 2) * TS
        out[bb, t0 : t0 + TS] = res.results[k]["out"].astype(np.float32)
    return out
